# revision 12
# baseline (speedup 1.0000x reference)
"""GAT layer (DiseaseGraphGAT) Trainium2 kernel, 8-way sharded over query rows.

Math (reference):
    s1 = emb @ attn[:D], s2 = emb @ attn[D:]          (N,)
    e  = leaky_relu(s1_i + s2_j, 0.2) masked by adj
    alpha = softmax(e, rows); out = alpha @ emb

Reformulation (exact, rank-1 separated). Row-scale invariance lets us
divide w_ij = exp(e_ij) by exp(s1_i):
    w_ij / exp(s1_i) = max(exp(-0.8 s1_i + 0.2 s2_j), exp(s2_j))
                     = max(A_i * B_j, q4_j)
with A = exp(-0.8 s1), B = exp(0.2 s2), q4 = exp(s2) all O(N), computed
on host. The full N x N weight matrix is max(rank-1, column-const) times
the adjacency mask -- no exp evaluation on device at all.

Device layout: j on partitions (adj uploaded pre-transposed as bf16), i
along free. Per j-block pair (2 x 128 j rows interleaved into one tile):
    1. DMA: adjT pair tile [128 j, 2*1024 i] bf16, one instruction
    2. DVE tensor_scalar (4x): m_h = (A_strip * B_p) max q4_p per half
    3. DVE/Pool tensor_tensor (2x): aw = m * adjT  (exact 0/keep mask)
    4. DVE/Pool: s = awL + awR  (pair-sum halves the z matmul count)
    5. PE: ps_num[d,i] += emb_jb.T @ aw_h ; ps_z[1,i] += ones.T @ s
No transposes (j already on partitions), no ACT pass, z rides PE on
pair-sums. Host does the tiny O(N) precompute and the final num/z
divide.
"""

import sys

sys.path.insert(0, "/opt/trn_rl_repo")

import numpy as np
import ml_dtypes

import concourse.bacc as bacc
import concourse.mybir as mybir
import concourse.tile as tile
from concourse.bass_utils import run_bass_kernel_spmd

N = 8192
D = 128
NCORES = 8
NI = N // NCORES               # 1024 query rows (i) per core
NJB = N // 128                 # 64 j-blocks of 128
NP = NJB // 2                  # 32 j-block pairs
IC = 512                       # i-chunk (matmul moving free dim)
NIC = NI // IC                 # 2 i-chunks
AD_BUFS = 8
M_BUFS = 4
AW_BUFS = 5
POOL_EVERY = 3                 # jb % POOL_EVERY == 1 -> mask-mult on Pool

_cache = {}


def _build_program(repeat=1, stages=("load", "m", "aw", "mm"), variant="contig",
                   hwloop=True):
    key = (repeat, tuple(stages), variant, hwloop)
    if key in _cache:
        return _cache[key]
    nc = bacc.Bacc("TRN2", target_bir_lowering=False, debug=False)
    adjt_shape = [N // 2, 2 * NI] if variant == "contig" else [N, NI]
    adjt_d = nc.declare_dram_parameter("adjt", adjt_shape, mybir.dt.bfloat16, isOutput=False)
    a_d = nc.declare_dram_parameter("astrip", [128, NI], mybir.dt.bfloat16, isOutput=False)
    btab_d = nc.declare_dram_parameter("btab", [128, NJB], mybir.dt.float32, isOutput=False)
    qtab_d = nc.declare_dram_parameter("qtab", [128, NJB], mybir.dt.float32, isOutput=False)
    emb_d = nc.declare_dram_parameter("embd", [128, NJB * D], mybir.dt.bfloat16, isOutput=False)
    numt_d = nc.declare_dram_parameter("numt", [D, NI], mybir.dt.float32, isOutput=True)
    z_d = nc.declare_dram_parameter("z", [1, NI], mybir.dt.float32, isOutput=True)

    with tile.TileContext(nc) as tc:
        with (
            tc.tile_pool(name="pre", bufs=1) as pre_pool,
            tc.tile_pool(name="adp", bufs=AD_BUFS) as adp,
            tc.tile_pool(name="mp", bufs=M_BUFS) as mp,
            tc.tile_pool(name="awp", bufs=AW_BUFS) as awp,
            tc.tile_pool(name="outp", bufs=2) as outp,
            tc.tile_pool(name="ps", bufs=4, space="PSUM") as ps,
        ):
          import contextlib
          with contextlib.ExitStack() as reps:
            if repeat > 1 and hwloop:
                reps.enter_context(tc.For_i(0, repeat, 1))
            elif repeat > 1:
                raise NotImplementedError("unrolled repeat removed")
            astrip = pre_pool.tile([128, NI], mybir.dt.bfloat16)
            nc.sync.dma_start(out=astrip[:], in_=a_d[:])
            btab = pre_pool.tile([128, NJB], mybir.dt.float32)
            nc.sync.dma_start(out=btab[:], in_=btab_d[:])
            qtab = pre_pool.tile([128, NJB], mybir.dt.float32)
            nc.sync.dma_start(out=qtab[:], in_=qtab_d[:])
            emb = pre_pool.tile([128, NJB * D], mybir.dt.bfloat16)
            nc.sync.dma_start(out=emb[:], in_=emb_d[:])
            ones = pre_pool.tile([128, 1], mybir.dt.bfloat16)
            nc.gpsimd.memset(ones[:], 1.0)

            ps_num = [ps.tile([D, IC], mybir.dt.float32, tag="psn", name=f"psn{_}") for _ in range(NIC)]
            ps_z = [ps.tile([1, IC], mybir.dt.float32, tag="psz", name=f"psz{_}") for _ in range(NIC)]

            for k in range(NP):
                adjt = adp.tile([128, 2 * NI], mybir.dt.bfloat16, tag="ad")
                if "load" in stages:
                    if variant == "contig":
                        nc.sync.dma_start(
                            out=adjt[:], in_=adjt_d[k * 128:(k + 1) * 128, :])
                    else:
                        src3 = adjt_d[k * 256:(k + 1) * 256, :].rearrange(
                            "(b p) i -> p b i", p=128)
                        dst3 = adjt[:].rearrange("p (b i) -> p b i", b=2)
                        nc.sync.dma_start(out=dst3, in_=src3)
                m = mp.tile([128, 2 * NI], mybir.dt.bfloat16, tag="m")
                if "m" in stages:
                    for h in range(2):
                        jb = 2 * k + h
                        nc.vector.tensor_scalar(
                            m[:, h * NI:(h + 1) * NI], astrip[:],
                            btab[:, jb:jb + 1], qtab[:, jb:jb + 1],
                            mybir.AluOpType.mult, mybir.AluOpType.max)
                aw = awp.tile([128, 2 * NI], mybir.dt.bfloat16, tag="aw")
                if "aw" in stages:
                    for h in range(2):
                        jb = 2 * k + h
                        eng = nc.gpsimd if jb % POOL_EVERY == 1 else nc.vector
                        eng.tensor_tensor(aw[:, h * NI:(h + 1) * NI],
                                          m[:, h * NI:(h + 1) * NI],
                                          adjt[:, h * NI:(h + 1) * NI],
                                          mybir.AluOpType.mult)
                if "mm" not in stages:
                    continue
                for h in range(2):
                    jb = 2 * k + h
                    first, last = jb == 0, jb == NJB - 1
                    for ic in range(NIC):
                        rhs = aw[:, h * NI + ic * IC: h * NI + (ic + 1) * IC]
                        nc.tensor.matmul(ps_num[ic][:],
                                         emb[:, jb * D:(jb + 1) * D], rhs,
                                         start=first, stop=last)
                        nc.tensor.matmul(ps_z[ic][:], ones[:], rhs,
                                         start=first, stop=last)

            if "mm" in stages:
                onum = outp.tile([D, NI], mybir.dt.float32, tag="on")
                oz = outp.tile([1, NI], mybir.dt.float32, tag="oz")
                for ic in range(NIC):
                    nc.scalar.copy(onum[:, ic * IC:(ic + 1) * IC], ps_num[ic][:])
                    nc.vector.tensor_copy(oz[:, ic * IC:(ic + 1) * IC], ps_z[ic][:])
                nc.sync.dma_start(out=numt_d[:], in_=onum[:])
                nc.sync.dma_start(out=z_d[:], in_=oz[:])

    nc.compile()
    _cache[key] = nc
    return nc


def prep_in_maps(adj: np.ndarray, emb: np.ndarray, attn: np.ndarray, variant="contig") -> list:
    bf16 = ml_dtypes.bfloat16
    emb64 = emb.astype(np.float64)
    s1 = (emb64 @ attn[:D, 0].astype(np.float64)).astype(np.float32)
    s2 = (emb64 @ attn[D:, 0].astype(np.float64)).astype(np.float32)

    a_full = np.exp(-0.8 * s1.astype(np.float64)).astype(np.float32)   # (N,)
    btab = np.ascontiguousarray(
        np.exp(0.2 * s2.astype(np.float64)).astype(np.float32).reshape(NJB, 128).T)
    qtab = np.ascontiguousarray(
        np.exp(s2.astype(np.float64)).astype(np.float32).reshape(NJB, 128).T)
    # emb_dev[p, jb*D + d] = emb[jb*128 + p, d]
    emb_dev = np.ascontiguousarray(
        emb.reshape(NJB, 128, D).transpose(1, 0, 2).reshape(128, NJB * D)
    ).astype(bf16)

    adj_bf = adj.astype(bf16)           # {0,1} exact in bf16
    in_maps = []
    for c in range(NCORES):
        rows = slice(c * NI, (c + 1) * NI)
        adjt = adj_bf[rows].T                                # (N, NI) view
        if variant == "contig":
            # pair layout: row p of tile k = [adjT[256k+p], adjT[256k+128+p]]
            adjt = np.ascontiguousarray(
                adjt.reshape(NP, 2, 128, NI).transpose(0, 2, 1, 3)
            ).reshape(N // 2, 2 * NI)
        else:
            adjt = np.ascontiguousarray(adjt)
        astrip = np.broadcast_to(
            a_full[rows].astype(bf16)[None, :], (128, NI)).copy()
        in_maps.append({
            "adjt": adjt,
            "astrip": astrip,
            "btab": btab,
            "qtab": qtab,
            "embd": emb_dev,
        })
    return in_maps


def kernel(adj: np.ndarray, emb: np.ndarray, attn: np.ndarray) -> np.ndarray:
    in_maps = prep_in_maps(adj, emb, attn)
    nc = _build_program()
    res = run_bass_kernel_spmd(nc, in_maps, core_ids=list(range(NCORES)))

    out = np.empty((N, D), np.float32)
    for c, r in enumerate(res.results):
        numt = r["numt"]          # (D, NI)
        z = r["z"]                # (1, NI)
        out[c * NI:(c + 1) * NI] = (numt / z).T
    return out


# revision 15
# speedup vs baseline: 1.1325x; 1.1325x over previous
"""GAT layer (DiseaseGraphGAT) Trainium2 kernel, 8-way sharded over query rows.

Math (reference):
    s1 = emb @ attn[:D], s2 = emb @ attn[D:]          (N,)
    e  = leaky_relu(s1_i + s2_j, 0.2) masked by adj
    alpha = softmax(e, rows); out = alpha @ emb

Reformulation (exact, rank-1 separated). Row-scale invariance lets us
divide w_ij = exp(e_ij) by exp(s1_i):
    w_ij / exp(s1_i) = max(exp(-0.8 s1_i + 0.2 s2_j), exp(s2_j))
                     = max(A_i * B_j, q4_j)
with A = exp(-0.8 s1), B = exp(0.2 s2), q4 = exp(s2) all O(N), computed
on host. The full N x N weight matrix is max(rank-1, column-const) times
the adjacency mask -- no exp evaluation on device at all.

Device layout: j on partitions (adj uploaded pre-transposed as bf16), i
along free. Per j-block pair (2 x 128 j rows interleaved into one tile):
    1. DMA: adjT pair tile [128 j, 2*1024 i] bf16, one instruction
    2. DVE tensor_scalar (4x): m_h = (A_strip * B_p) max q4_p per half
    3. DVE/Pool tensor_tensor (2x): aw = m * adjT  (exact 0/keep mask)
    4. DVE/Pool: s = awL + awR  (pair-sum halves the z matmul count)
    5. PE: ps_num[d,i] += emb_jb.T @ aw_h ; ps_z[1,i] += ones.T @ s
No transposes (j already on partitions), no ACT pass, z rides PE on
pair-sums. Host does the tiny O(N) precompute and the final num/z
divide.
"""

import sys

sys.path.insert(0, "/opt/trn_rl_repo")

import numpy as np
import ml_dtypes

import concourse.bacc as bacc
import concourse.mybir as mybir
import concourse.tile as tile
from concourse.bass_utils import run_bass_kernel_spmd

N = 8192
D = 128
NCORES = 8
NI = N // NCORES               # 1024 query rows (i) per core
NJB = N // 128                 # 64 j-blocks of 128
NP = NJB // 2                  # 32 j-block pairs
IC = 512                       # i-chunk (matmul moving free dim)
NIC = NI // IC                 # 2 i-chunks
AD_BUFS = 8
M_BUFS = 4
AW_BUFS = 7
POOL_EVERY = 10**9             # jb % POOL_EVERY == 1 -> mask-mult on Pool (disabled: Pool is slow on HW)

_cache = {}


def _build_program(repeat=1, stages=("load", "m", "aw", "mm"), variant="contig",
                   hwloop=True, pool_every=None):
    pe_ = POOL_EVERY if pool_every is None else pool_every
    key = (repeat, tuple(stages), variant, hwloop, pe_)
    if key in _cache:
        return _cache[key]
    nc = bacc.Bacc("TRN2", target_bir_lowering=False, debug=False)
    adjt_shape = [N // 2, 2 * NI] if variant == "contig" else [N, NI]
    adjt_d = nc.declare_dram_parameter("adjt", adjt_shape, mybir.dt.bfloat16, isOutput=False)
    a_d = nc.declare_dram_parameter("astrip", [128, NI], mybir.dt.bfloat16, isOutput=False)
    btab_d = nc.declare_dram_parameter("btab", [128, NJB], mybir.dt.float32, isOutput=False)
    qtab_d = nc.declare_dram_parameter("qtab", [128, NJB], mybir.dt.float32, isOutput=False)
    emb_d = nc.declare_dram_parameter("embd", [128, NJB * D], mybir.dt.bfloat16, isOutput=False)
    numt_d = nc.declare_dram_parameter("numt", [D, NI], mybir.dt.float32, isOutput=True)
    z_d = nc.declare_dram_parameter("z", [1, NI], mybir.dt.float32, isOutput=True)

    with tile.TileContext(nc) as tc:
        with (
            tc.tile_pool(name="pre", bufs=1) as pre_pool,
            tc.tile_pool(name="adp", bufs=AD_BUFS) as adp,
            tc.tile_pool(name="mp", bufs=M_BUFS) as mp,
            tc.tile_pool(name="awp", bufs=AW_BUFS) as awp,
            tc.tile_pool(name="outp", bufs=2) as outp,
            tc.tile_pool(name="ps", bufs=4, space="PSUM") as ps,
        ):
          import contextlib
          with contextlib.ExitStack() as reps:
            if repeat > 1 and hwloop:
                reps.enter_context(tc.For_i(0, repeat, 1))
            elif repeat > 1:
                raise NotImplementedError("unrolled repeat removed")
            astrip = pre_pool.tile([128, NI], mybir.dt.bfloat16)
            nc.sync.dma_start(out=astrip[:], in_=a_d[:])
            btab = pre_pool.tile([128, NJB], mybir.dt.float32)
            nc.sync.dma_start(out=btab[:], in_=btab_d[:])
            qtab = pre_pool.tile([128, NJB], mybir.dt.float32)
            nc.sync.dma_start(out=qtab[:], in_=qtab_d[:])
            emb = pre_pool.tile([128, NJB * D], mybir.dt.bfloat16)
            nc.sync.dma_start(out=emb[:], in_=emb_d[:])
            ones = pre_pool.tile([128, 1], mybir.dt.bfloat16)
            nc.gpsimd.memset(ones[:], 1.0)

            ps_num = [ps.tile([D, IC], mybir.dt.float32, tag="psn", name=f"psn{_}") for _ in range(NIC)]
            ps_z = [ps.tile([1, IC], mybir.dt.float32, tag="psz", name=f"psz{_}") for _ in range(NIC)]

            for k in range(NP):
                adjt = adp.tile([128, 2 * NI], mybir.dt.bfloat16, tag="ad")
                if "load" in stages:
                    if variant == "contig":
                        nc.sync.dma_start(
                            out=adjt[:], in_=adjt_d[k * 128:(k + 1) * 128, :])
                    else:
                        src3 = adjt_d[k * 256:(k + 1) * 256, :].rearrange(
                            "(b p) i -> p b i", p=128)
                        dst3 = adjt[:].rearrange("p (b i) -> p b i", b=2)
                        nc.sync.dma_start(out=dst3, in_=src3)
                m = mp.tile([128, 2 * NI], mybir.dt.bfloat16, tag="m")
                if "m" in stages:
                    for h in range(2):
                        jb = 2 * k + h
                        nc.vector.tensor_scalar(
                            m[:, h * NI:(h + 1) * NI], astrip[:],
                            btab[:, jb:jb + 1], qtab[:, jb:jb + 1],
                            mybir.AluOpType.mult, mybir.AluOpType.max)
                aw = awp.tile([128, 2 * NI], mybir.dt.bfloat16, tag="aw")
                if "aw" in stages:
                    eng = nc.gpsimd if k % pe_ == 1 else nc.vector
                    eng.tensor_tensor(aw[:], m[:], adjt[:],
                                      mybir.AluOpType.mult)
                if "mm" not in stages:
                    continue
                for h in range(2):
                    jb = 2 * k + h
                    first, last = jb == 0, jb == NJB - 1
                    for ic in range(NIC):
                        rhs = aw[:, h * NI + ic * IC: h * NI + (ic + 1) * IC]
                        nc.tensor.matmul(ps_num[ic][:],
                                         emb[:, jb * D:(jb + 1) * D], rhs,
                                         start=first, stop=last)
                        if "noz" not in stages:
                            nc.tensor.matmul(ps_z[ic][:], ones[:], rhs,
                                             start=first, stop=last)

            if "mm" in stages:
                onum = outp.tile([D, NI], mybir.dt.float32, tag="on")
                oz = outp.tile([1, NI], mybir.dt.float32, tag="oz")
                for ic in range(NIC):
                    nc.scalar.copy(onum[:, ic * IC:(ic + 1) * IC], ps_num[ic][:])
                    if "noz" not in stages:
                        nc.vector.tensor_copy(oz[:, ic * IC:(ic + 1) * IC], ps_z[ic][:])
                if "noz" in stages:
                    nc.vector.memset(oz[:], 1.0)
                nc.sync.dma_start(out=numt_d[:], in_=onum[:])
                nc.sync.dma_start(out=z_d[:], in_=oz[:])

    nc.compile()
    _cache[key] = nc
    return nc


def prep_in_maps(adj: np.ndarray, emb: np.ndarray, attn: np.ndarray, variant="contig") -> list:
    bf16 = ml_dtypes.bfloat16
    emb64 = emb.astype(np.float64)
    s1 = (emb64 @ attn[:D, 0].astype(np.float64)).astype(np.float32)
    s2 = (emb64 @ attn[D:, 0].astype(np.float64)).astype(np.float32)

    a_full = np.exp(-0.8 * s1.astype(np.float64)).astype(np.float32)   # (N,)
    btab = np.ascontiguousarray(
        np.exp(0.2 * s2.astype(np.float64)).astype(np.float32).reshape(NJB, 128).T)
    qtab = np.ascontiguousarray(
        np.exp(s2.astype(np.float64)).astype(np.float32).reshape(NJB, 128).T)
    # emb_dev[p, jb*D + d] = emb[jb*128 + p, d]
    emb_dev = np.ascontiguousarray(
        emb.reshape(NJB, 128, D).transpose(1, 0, 2).reshape(128, NJB * D)
    ).astype(bf16)

    adj_bf = adj.astype(bf16)           # {0,1} exact in bf16
    in_maps = []
    for c in range(NCORES):
        rows = slice(c * NI, (c + 1) * NI)
        adjt = adj_bf[rows].T                                # (N, NI) view
        if variant == "contig":
            # pair layout: row p of tile k = [adjT[256k+p], adjT[256k+128+p]]
            adjt = np.ascontiguousarray(
                adjt.reshape(NP, 2, 128, NI).transpose(0, 2, 1, 3)
            ).reshape(N // 2, 2 * NI)
        else:
            adjt = np.ascontiguousarray(adjt)
        astrip = np.broadcast_to(
            a_full[rows].astype(bf16)[None, :], (128, NI)).copy()
        in_maps.append({
            "adjt": adjt,
            "astrip": astrip,
            "btab": btab,
            "qtab": qtab,
            "embd": emb_dev,
        })
    return in_maps


def kernel(adj: np.ndarray, emb: np.ndarray, attn: np.ndarray) -> np.ndarray:
    in_maps = prep_in_maps(adj, emb, attn)
    nc = _build_program()
    res = run_bass_kernel_spmd(nc, in_maps, core_ids=list(range(NCORES)))

    out = np.empty((N, D), np.float32)
    for c, r in enumerate(res.results):
        numt = r["numt"]          # (D, NI)
        z = r["z"]                # (1, NI)
        out[c * NI:(c + 1) * NI] = (numt / z).T
    return out


# revision 17
# speedup vs baseline: 2.0693x; 1.8272x over previous
"""GAT layer (DiseaseGraphGAT) Trainium2 kernel, 8-way sharded over query rows.

Math (reference):
    s1 = emb @ attn[:D], s2 = emb @ attn[D:]          (N,)
    e  = leaky_relu(s1_i + s2_j, 0.2) masked by adj
    alpha = softmax(e, rows); out = alpha @ emb

Reformulation (exact, rank-1 separated). Row-scale invariance lets us
divide w_ij = exp(e_ij) by exp(s1_i):
    w_ij / exp(s1_i) = max(exp(-0.8 s1_i + 0.2 s2_j), exp(s2_j))
                     = max(A_i * B_j, q4_j)
with A = exp(-0.8 s1), B = exp(0.2 s2), q4 = exp(s2) all O(N), computed
on host. The full N x N weight matrix is max(rank-1, column-const) times
the adjacency mask -- no exp evaluation on device at all.

Device layout: j on partitions (adj uploaded pre-transposed as bf16), i
along free. Per j-block pair (2 x 128 j rows interleaved into one tile):
    1. DMA: adjT pair tile [128 j, 2*1024 i] bf16, one instruction
    2. DVE tensor_scalar (4x): m_h = (A_strip * B_p) max q4_p per half
    3. DVE/Pool tensor_tensor (2x): aw = m * adjT  (exact 0/keep mask)
    4. DVE/Pool: s = awL + awR  (pair-sum halves the z matmul count)
    5. PE: ps_num[d,i] += emb_jb.T @ aw_h ; ps_z[1,i] += ones.T @ s
No transposes (j already on partitions), no ACT pass, z rides PE on
pair-sums. Host does the tiny O(N) precompute and the final num/z
divide.
"""

import sys

sys.path.insert(0, "/opt/trn_rl_repo")

import numpy as np
import ml_dtypes

import concourse.bacc as bacc
import concourse.mybir as mybir
import concourse.tile as tile
from concourse.bass_utils import run_bass_kernel_spmd

N = 8192
D = 128
NCORES = 8
NI = N // NCORES               # 1024 query rows (i) per core
NJB = N // 128                 # 64 j-blocks of 128
NP = NJB // 2                  # 32 j-block pairs
IC = 512                       # i-chunk (matmul moving free dim)
NIC = NI // IC                 # 2 i-chunks
AD_BUFS = 8
M_BUFS = 4
AW_BUFS = 7
POOL_EVERY = 10**9             # jb % POOL_EVERY == 1 -> mask-mult on Pool (disabled: Pool is slow on HW)

_cache = {}


def _build_program(repeat=1, stages=("load", "m", "aw", "mm", "zlast"), variant="contig",
                   hwloop=True, pool_every=None):
    pe_ = POOL_EVERY if pool_every is None else pool_every
    key = (repeat, tuple(stages), variant, hwloop, pe_)
    if key in _cache:
        return _cache[key]
    nc = bacc.Bacc("TRN2", target_bir_lowering=False, debug=False)
    adjt_shape = [N // 2, 2 * NI] if variant == "contig" else [N, NI]
    adjt_d = nc.declare_dram_parameter("adjt", adjt_shape, mybir.dt.bfloat16, isOutput=False)
    a_d = nc.declare_dram_parameter("astrip", [128, NI], mybir.dt.bfloat16, isOutput=False)
    btab_d = nc.declare_dram_parameter("btab", [128, NJB], mybir.dt.float32, isOutput=False)
    qtab_d = nc.declare_dram_parameter("qtab", [128, NJB], mybir.dt.float32, isOutput=False)
    emb_d = nc.declare_dram_parameter("embd", [128, NJB * D], mybir.dt.bfloat16, isOutput=False)
    numt_d = nc.declare_dram_parameter("numt", [D, NI], mybir.dt.float32, isOutput=True)
    z_d = nc.declare_dram_parameter("z", [1, NI], mybir.dt.float32, isOutput=True)

    with tile.TileContext(nc) as tc:
        with (
            tc.tile_pool(name="pre", bufs=1) as pre_pool,
            tc.tile_pool(name="adp", bufs=AD_BUFS) as adp,
            tc.tile_pool(name="mp", bufs=M_BUFS) as mp,
            tc.tile_pool(name="awp", bufs=AW_BUFS) as awp,
            tc.tile_pool(name="outp", bufs=2) as outp,
            tc.tile_pool(name="ps", bufs=4, space="PSUM") as ps,
        ):
          import contextlib
          with contextlib.ExitStack() as reps:
            if repeat > 1 and hwloop:
                reps.enter_context(tc.For_i(0, repeat, 1))
            elif repeat > 1:
                raise NotImplementedError("unrolled repeat removed")
            astrip = pre_pool.tile([128, NI], mybir.dt.bfloat16)
            nc.sync.dma_start(out=astrip[:], in_=a_d[:])
            btab = pre_pool.tile([128, NJB], mybir.dt.float32)
            nc.sync.dma_start(out=btab[:], in_=btab_d[:])
            qtab = pre_pool.tile([128, NJB], mybir.dt.float32)
            nc.sync.dma_start(out=qtab[:], in_=qtab_d[:])
            emb = pre_pool.tile([128, NJB * D], mybir.dt.bfloat16)
            nc.sync.dma_start(out=emb[:], in_=emb_d[:])
            ones = pre_pool.tile([128, 1], mybir.dt.bfloat16)
            nc.gpsimd.memset(ones[:], 1.0)

            ps_num = [ps.tile([D, IC], mybir.dt.float32, tag="psn", name=f"psn{_}") for _ in range(NIC)]
            ps_z = [ps.tile([1, IC], mybir.dt.float32, tag="psz", name=f"psz{_}") for _ in range(NIC)]

            for k in range(NP):
                adjt = adp.tile([128, 2 * NI], mybir.dt.bfloat16, tag="ad")
                if "load" in stages:
                    if variant == "contig":
                        nc.sync.dma_start(
                            out=adjt[:], in_=adjt_d[k * 128:(k + 1) * 128, :])
                    else:
                        src3 = adjt_d[k * 256:(k + 1) * 256, :].rearrange(
                            "(b p) i -> p b i", p=128)
                        dst3 = adjt[:].rearrange("p (b i) -> p b i", b=2)
                        nc.sync.dma_start(out=dst3, in_=src3)
                m = mp.tile([128, 2 * NI], mybir.dt.bfloat16, tag="m")
                if "m" in stages:
                    for h in range(2):
                        jb = 2 * k + h
                        nc.vector.tensor_scalar(
                            m[:, h * NI:(h + 1) * NI], astrip[:],
                            btab[:, jb:jb + 1], qtab[:, jb:jb + 1],
                            mybir.AluOpType.mult, mybir.AluOpType.max)
                aw = awp.tile([128, 2 * NI], mybir.dt.bfloat16, tag="aw")
                if "aw" in stages:
                    eng = nc.gpsimd if k % pe_ == 1 else nc.vector
                    eng.tensor_tensor(aw[:], m[:], adjt[:],
                                      mybir.AluOpType.mult)
                if "mm" not in stages:
                    continue
                def chunk(h, ic):
                    return aw[:, h * NI + ic * IC: h * NI + (ic + 1) * IC]
                if "zlast" in stages:
                    for h in range(2):
                        jb = 2 * k + h
                        first, last = jb == 0, jb == NJB - 1
                        for ic in range(NIC):
                            nc.tensor.matmul(ps_num[ic][:],
                                             emb[:, jb * D:(jb + 1) * D],
                                             chunk(h, ic),
                                             start=first, stop=last)
                    for h in range(2):
                        jb = 2 * k + h
                        first, last = jb == 0, jb == NJB - 1
                        for ic in range(NIC):
                            nc.tensor.matmul(ps_z[ic][:], ones[:], chunk(h, ic),
                                             start=first, stop=last)
                else:
                    for h in range(2):
                        jb = 2 * k + h
                        first, last = jb == 0, jb == NJB - 1
                        for ic in range(NIC):
                            rhs = chunk(h, ic)
                            nc.tensor.matmul(ps_num[ic][:],
                                             emb[:, jb * D:(jb + 1) * D], rhs,
                                             start=first, stop=last)
                            if "noz" not in stages:
                                nc.tensor.matmul(ps_z[ic][:], ones[:], rhs,
                                                 start=first, stop=last)

            if "mm" in stages:
                onum = outp.tile([D, NI], mybir.dt.float32, tag="on")
                oz = outp.tile([1, NI], mybir.dt.float32, tag="oz")
                for ic in range(NIC):
                    nc.scalar.copy(onum[:, ic * IC:(ic + 1) * IC], ps_num[ic][:])
                    if "noz" not in stages:
                        nc.vector.tensor_copy(oz[:, ic * IC:(ic + 1) * IC], ps_z[ic][:])
                if "noz" in stages:
                    nc.vector.memset(oz[:], 1.0)
                nc.sync.dma_start(out=numt_d[:], in_=onum[:])
                nc.sync.dma_start(out=z_d[:], in_=oz[:])

    nc.compile()
    _cache[key] = nc
    return nc


def prep_in_maps(adj: np.ndarray, emb: np.ndarray, attn: np.ndarray, variant="contig") -> list:
    bf16 = ml_dtypes.bfloat16
    emb64 = emb.astype(np.float64)
    s1 = (emb64 @ attn[:D, 0].astype(np.float64)).astype(np.float32)
    s2 = (emb64 @ attn[D:, 0].astype(np.float64)).astype(np.float32)

    a_full = np.exp(-0.8 * s1.astype(np.float64)).astype(np.float32)   # (N,)
    btab = np.ascontiguousarray(
        np.exp(0.2 * s2.astype(np.float64)).astype(np.float32).reshape(NJB, 128).T)
    qtab = np.ascontiguousarray(
        np.exp(s2.astype(np.float64)).astype(np.float32).reshape(NJB, 128).T)
    # emb_dev[p, jb*D + d] = emb[jb*128 + p, d]
    emb_dev = np.ascontiguousarray(
        emb.reshape(NJB, 128, D).transpose(1, 0, 2).reshape(128, NJB * D)
    ).astype(bf16)

    adj_bf = adj.astype(bf16)           # {0,1} exact in bf16
    in_maps = []
    for c in range(NCORES):
        rows = slice(c * NI, (c + 1) * NI)
        adjt = adj_bf[rows].T                                # (N, NI) view
        if variant == "contig":
            # pair layout: row p of tile k = [adjT[256k+p], adjT[256k+128+p]]
            adjt = np.ascontiguousarray(
                adjt.reshape(NP, 2, 128, NI).transpose(0, 2, 1, 3)
            ).reshape(N // 2, 2 * NI)
        else:
            adjt = np.ascontiguousarray(adjt)
        astrip = np.broadcast_to(
            a_full[rows].astype(bf16)[None, :], (128, NI)).copy()
        in_maps.append({
            "adjt": adjt,
            "astrip": astrip,
            "btab": btab,
            "qtab": qtab,
            "embd": emb_dev,
        })
    return in_maps


def kernel(adj: np.ndarray, emb: np.ndarray, attn: np.ndarray) -> np.ndarray:
    in_maps = prep_in_maps(adj, emb, attn)
    nc = _build_program()
    res = run_bass_kernel_spmd(nc, in_maps, core_ids=list(range(NCORES)))

    out = np.empty((N, D), np.float32)
    for c, r in enumerate(res.results):
        numt = r["numt"]          # (D, NI)
        z = r["z"]                # (1, NI)
        out[c * NI:(c + 1) * NI] = (numt / z).T
    return out
